# revision 12
# baseline (speedup 1.0000x reference)
"""XCA (cross-covariance) attention block on 8 TRN2 NeuronCores.

Sharding: 8 cores = 4 batches x 2 image-row-halves. Each core computes the
full pipeline (1x1 qkv conv -> 3x3 depthwise conv -> per-head channel
attention -> 1x1 proj) for its (batch, row-half). Gram matrices + sq-norms
need global spatial sums -> pairwise all-reduce of a tiny f32 buffer.

v2: right-pad-only frame layout (even offsets -> DVE 2x STT taps), center
tap fused into the psum evacuation, tap-major PE matmul rounds, attention
folded into the projection weights (M = Wp @ A), balanced engine split.
"""

import numpy as np
import ml_dtypes

B, C = 4, 384
HEADS, CHD = 8, 48
WP = 130            # row pitch: cols 0..127 valid, 128/129 zero pad
HR = 32             # valid rows per half
XROWS = HR + 2      # qkv rows per half (halo)
SP_H = XROWS * WP   # 4420
DWH = HR * WP       # 4160 dw output window (rows 1..32, cols 0..129)
LEAD = 2            # lead zero cols in qkv_pre (for tap delta=-131)
QPW = LEAD + SP_H + 4
SP8 = 4432          # fp8 x row-pitch per channel block (16B aligned)
DWBASE = LEAD + WP  # qkv_pre index of output (row 1, x 0) = 132 (even)
NT = 8192
BF16 = ml_dtypes.bfloat16

PE_TAPS = (2, 3, 5, 6, 8)   # odd-offset taps on TensorE (diag matmuls)
SC_TAP = 0                  # (-1,-1): ScalarE scaled copy + DVE add
DVE_TAPS = (1, 7)           # (dy=-1/+1, dx=0): even offsets -> STT 2x
FUSED_TAP = 4               # center tap fused into psum->sbuf STT

_CACHE = {}


def _tiles(total, step=512):
    out = []
    s = 0
    while s < total:
        out.append((s, min(step, total - s)))
        s += step
    return out


def _delta(k9):
    return WP * (k9 // 3 - 1) + (k9 % 3 - 1)


def _build_body(nc, tc, tens):
    import concourse.mybir as mybir
    dt = mybir.dt
    Alu = mybir.AluOpType
    Act = mybir.ActivationFunctionType
    AX = mybir.AxisListType

    xd, wqd, dtd, wpd, idd, mkd, tpd, dsd, outd, x8d, w8d = tens

    cst = tc.alloc_tile_pool(name="cst", bufs=1)
    xp = tc.alloc_tile_pool(name="xp", bufs=1)
    qp = tc.alloc_tile_pool(name="qp", bufs=2)
    tmpp = tc.alloc_tile_pool(name="tmp", bufs=2)
    dwp = tc.alloc_tile_pool(name="dwp", bufs=1)
    wk = tc.alloc_tile_pool(name="wk", bufs=2)
    mmp = tc.alloc_tile_pool(name="mm", bufs=1, space="PSUM")
    gtp = tc.alloc_tile_pool(name="gt", bufs=1, space="PSUM")
    drp = tc.alloc_tile_pool(name="dr", bufs=1, space="DRAM")

    # ---- constants in ----
    wq = cst.tile([128, 9 * 384], dt.bfloat16, tag="wq")
    nc.sync.dma_start(out=wq[:], in_=wqd.ap())
    wp_sb = cst.tile([128, 3 * 384], dt.bfloat16, tag="wp")
    nc.sync.dma_start(out=wp_sb[:], in_=wpd.ap())
    identb = cst.tile([128, 128], dt.bfloat16, tag="id")
    nc.sync.dma_start(out=identb[:], in_=idd.ap())
    mask_sb = cst.tile([128, 3 * 384], dt.float32, tag="mk")
    nc.sync.dma_start(out=mask_sb[:], in_=mkd.ap())
    tempc_sb = cst.tile([128, 4], dt.float32, tag="tc")
    nc.sync.dma_start(out=tempc_sb[:], in_=tpd.ap())
    dwsc_sb = cst.tile([128, 81], dt.float32, tag="ds")
    nc.sync.dma_start(out=dwsc_sb[:], in_=dsd.ap())
    wq8 = cst.tile([128, 6 * 384], dt.float8e4, tag="wq8")
    nc.sync.dma_start(out=wq8[:], in_=w8d.ap())

    nsum = cst.tile([128, 12], dt.float32, tag="ns")

    def load_x(h):
        xs = []
        for ch in range(3):
            t = xp.tile([128, SP_H], dt.bfloat16, tag=f"x{ch}", name=f"x{ch}")
            nc.sync.dma_start(
                out=t[:], in_=xd.ap()[ch, :, HR * h * WP: HR * h * WP + SP_H])
            xs.append(t)
        x8 = xp.tile([128, 3 * SP8], dt.float8e4, tag="x8", name="x8")
        for ch in range(3):
            nc.sync.dma_start(
                out=x8[:, ch * SP8: ch * SP8 + SP_H],
                in_=x8d.ap()[ch, :, HR * h * WP: HR * h * WP + SP_H])
        return xs, x8

    def conv_block(blk, xs, dwt, nslot, vtags=False, x8=None):
        """1x1 qkv conv + 3x3 depthwise for channel block blk -> dwt."""
        dtt = wk.tile([128, 5 * 128], dt.bfloat16, tag="dt", name="dtt")
        nc.sync.dma_start(out=dtt[:], in_=dtd.ap()[:, blk * 640:(blk + 1) * 640])
        qpre = qp.tile([128, QPW], dt.bfloat16, tag="qp", name="qpre")
        nc.vector.memset(qpre[:, 0:LEAD], 0.0)
        nc.vector.memset(qpre[:, LEAD + SP_H:QPW], 0.0)
        x8r = x8[:].rearrange("p (c n) -> p c n", c=3) if x8 is not None else None
        for (s, n) in _tiles(SP_H):
            ps = mmp.tile([128, 512], dt.float32, tag="qk", bufs=2, name="psq")
            if x8 is not None:
                w8r = wq8[:].rearrange("p (b c m) -> p b c m", b=6, m=128)
                nc.tensor.matmul(
                    ps[:, :n], lhsT=w8r[:, blk, 0:2, :],
                    rhs=x8r[:, 0:2, s:s + n], start=True, stop=False,
                    perf_mode=mybir.MatmulPerfMode.DoubleRow)
                nc.tensor.matmul(
                    ps[:, :n], lhsT=w8r[:, blk, 2, :],
                    rhs=x8r[:, 2, s:s + n], start=False, stop=True)
                nc.scalar.activation(qpre[:, LEAD + s: LEAD + s + n],
                                     ps[:, :n], Act.Copy, scale=1.0 / 64.0)
            else:
                for ch in range(3):
                    nc.tensor.matmul(
                        ps[:, :n],
                        lhsT=wq[:, blk * 384 + ch * 128: blk * 384 + (ch + 1) * 128],
                        rhs=xs[ch][:, s:s + n],
                        start=(ch == 0), stop=(ch == 2))
                nc.scalar.activation(qpre[:, LEAD + s: LEAD + s + n],
                                     ps[:, :n], Act.Copy)
        # PE taps (tap-major within each 512 chunk) + fused evac/center-tap
        for ci, (s, n) in enumerate(_tiles(DWH)):
            if vtags:
                ps = gtp.tile([128, 512], dt.float32, tag=f"g{ci % 3}",
                              name="psd")
            else:
                ps = mmp.tile([128, 512], dt.float32, tag="dw", bufs=3,
                              name="psd")
            for ti, k9 in enumerate(PE_TAPS):
                d = _delta(k9)
                nc.tensor.matmul(
                    ps[:, :n],
                    lhsT=dtt[:, ti * 128:(ti + 1) * 128],
                    rhs=qpre[:, DWBASE + s + d: DWBASE + s + d + n],
                    start=(ti == 0), stop=(ti == len(PE_TAPS) - 1))
            nc.vector.scalar_tensor_tensor(
                dwt[:, s:s + n], qpre[:, DWBASE + s: DWBASE + s + n],
                dwsc_sb[:, blk * 9 + FUSED_TAP: blk * 9 + FUSED_TAP + 1],
                ps[:, :n], op0=Alu.mult, op1=Alu.add)
        for k9 in DVE_TAPS:
            d = _delta(k9)
            tt = tmpp.tile([128, DWH], dt.bfloat16, tag="tt", name="tt",
                           bufs=1)
            nc.vector.tensor_scalar_mul(
                tt[:], qpre[:, DWBASE + d: DWBASE + d + DWH],
                dwsc_sb[:, blk * 9 + k9: blk * 9 + k9 + 1])
            nc.vector.tensor_add(dwt[:], dwt[:], tt[:])
        tm = tmpp.tile([128, DWH], dt.bfloat16, tag="tm", name="tm")
        nc.scalar.activation(
            tm[:], qpre[:, DWBASE - 131: DWBASE - 131 + DWH], Act.Copy,
            scale=dwsc_sb[:, blk * 9 + SC_TAP: blk * 9 + SC_TAP + 1])
        nc.vector.tensor_add(dwt[:], dwt[:], tm[:])
        if nslot is not None:
            no = tmpp.tile([128, HR * 128], dt.bfloat16, tag="tm", name="no")
            nc.scalar.activation(
                no[:].rearrange("p (r c) -> p r c", r=HR),
                dwt[:].rearrange("p (r c) -> p r c", c=WP)[:, :, 0:128],
                Act.Square, accum_out=nsum[:, nslot:nslot + 1])

    def phase_b(h, dq, dk, gt):
        # transposes + gram for half h; psum from the "dw" tag only, so the
        # next half's qkv matmuls ("qk" tag) can overlap.
        for j in range(HR):
            qtp = mmp.tile([128, 384], dt.bfloat16, tag="dw", bufs=3, name="qtp")
            ktp = mmp.tile([128, 384], dt.bfloat16, tag="dw", bufs=3, name="ktp")
            for b in range(3):
                nc.tensor.transpose(
                    qtp[:, b * 128:(b + 1) * 128],
                    dq[b][:, j * WP: j * WP + 128].opt(), identb[:])
                nc.tensor.transpose(
                    ktp[:, b * 128:(b + 1) * 128],
                    dk[b][:, j * WP: j * WP + 128].opt(), identb[:])
            qts = wk.tile([128, 768], dt.bfloat16, tag="qts", bufs=2, name="qts")
            if j % 2 == 0:
                nc.scalar.activation(qts[:, 0:384], qtp[:], Act.Copy)
                nc.vector.tensor_copy(qts[:, 384:768], ktp[:])
            else:
                nc.vector.tensor_copy(qts[:, 0:384], qtp[:])
                nc.scalar.activation(qts[:, 384:768], ktp[:], Act.Copy)
            for i in range(3):
                nc.tensor.matmul(
                    gt[i][:],
                    lhsT=qts[:, 384 + i * 128: 384 + (i + 1) * 128],
                    rhs=qts[:, 0:384],
                    start=(j == 0), stop=(j == HR - 1))

    # ---------- main: per half: q,k blocks -> phase B -> collective, v ----------
    dv = {}
    g_half = []
    for h in range(2):
        xs, x8 = load_x(h)
        dq = [dwp.tile([128, DWH], dt.bfloat16, tag=f"d{i}", name=f"dq{i}")
              for i in range(3)]
        dk = [dwp.tile([128, DWH], dt.bfloat16, tag=f"d{3 + i}", name=f"dk{i}")
              for i in range(3)]
        for i in range(3):
            conv_block(i, xs, dq[i], h * 6 + i, x8=x8)
            conv_block(3 + i, xs, dk[i], h * 6 + 3 + i, x8=x8)
        gt = [gtp.tile([128, 384], dt.float32, tag=f"g{i}", name=f"gt{i}")
              for i in range(3)]
        phase_b(h, dq, dk, gt)
        # per-half pairwise all-reduce of gram + sq-norms (overlaps with the
        # v blocks / next half)
        cc_sb = cst.tile([128, 1158], dt.float32, tag=f"cc{h}", name="cc_sb")
        for i in range(3):
            nc.vector.tensor_copy(cc_sb[:, 384 * i:384 * (i + 1)], gt[i][:])
        nc.vector.tensor_copy(cc_sb[:, 1152:1158], nsum[:, h * 6:h * 6 + 6])
        cc_in = drp.tile([128, 1158], dt.float32, tag=f"ccin{h}", name="cc_in")
        cc_out = drp.tile([128, 1158], dt.float32, tag=f"ccout{h}", name="cc_out")
        nc.gpsimd.dma_start(out=cc_in[:], in_=cc_sb[:])
        nc.gpsimd.collective_compute(
            "AllReduce", Alu.add,
            replica_groups=[[0, 1], [2, 3], [4, 5], [6, 7]],
            ins=[cc_in.opt()], outs=[cc_out.opt()])
        gh = cst.tile([128, 1158], dt.float32, tag=f"gh{h}", name="g_half")
        nc.gpsimd.dma_start(out=gh[:], in_=cc_out[:])
        g_half.append(gh)
        # v blocks for this half (h0 -> fresh tags d6-d8; h1 -> reuse d0-d2,
        # whose q tiles are dead after phase_b(h1))
        for i in range(3):
            t = dwp.tile([128, DWH], dt.bfloat16,
                         tag=(f"d{6 + i}" if h == 0 else f"d{i}"),
                         name=f"dv{i}_{h}")
            conv_block(6 + i, xs, t, None, vtags=True)
            dv[(h, i)] = t

    g_sb = g_half[0]
    nc.vector.tensor_add(g_sb[:], g_half[0][:], g_half[1][:])

    # ---------- softmax (tiny) ----------
    rt = cst.tile([128, 8], dt.float32, tag="rt")
    nc.scalar.activation(rt[:, 0:6], g_sb[:, 1152:1158], Act.Sqrt)
    nc.vector.tensor_scalar_max(rt[:, 0:6], rt[:, 0:6], 1e-12)
    inv = cst.tile([128, 8], dt.float32, tag="inv")
    nc.vector.reciprocal(inv[:, 0:6], rt[:, 0:6])

    gts_t = [cst.tile([128, 384], dt.bfloat16, tag=f"gs{i}", name=f"gsb{i}")
             for i in range(3)]
    for i in range(3):  # row-scale G^T (rows = k channels, block i) by inv_k
        nc.vector.tensor_scalar_mul(
            gts_t[i][:], g_sb[:, 384 * i:384 * (i + 1)], inv[:, 3 + i:4 + i])
    gs_ps = [gtp.tile([128, 384], dt.bfloat16, tag=f"g{j}", name=f"gsp{j}")
             for j in range(3)]
    for j in range(3):
        for i in range(3):
            nc.tensor.transpose(
                gs_ps[j][:, i * 128:(i + 1) * 128],
                gts_t[i][:, j * 128:(j + 1) * 128], identb[:])
    attn_t = [cst.tile([128, 384], dt.bfloat16, tag=f"at{j}", name=f"attn{j}")
              for j in range(3)]
    sums = cst.tile([128, 4], dt.float32, tag="sm")
    for j in range(3):
        zf = wk.tile([128, 384], dt.float32, tag="zf", name="zf", bufs=1)
        nc.vector.tensor_scalar_mul(zf[:], gs_ps[j][:], inv[:, j:j + 1])
        z = wk.tile([128, 384], dt.float32, tag="zz", name="zz", bufs=1)
        nc.vector.scalar_tensor_tensor(
            z[:], zf[:], tempc_sb[:, j:j + 1],
            mask_sb[:, 384 * j:384 * (j + 1)],
            op0=Alu.mult, op1=Alu.add)
        negmax = wk.tile([128, 1], dt.float32, tag="nm", name="nm")
        nc.vector.tensor_reduce(negmax[:], z[:], AX.X, Alu.max, negate=True)
        nc.scalar.activation(
            attn_t[j][:], z[:], Act.Exp, bias=negmax[:, 0:1],
            accum_out=sums[:, j:j + 1])
    invs = cst.tile([128, 4], dt.float32, tag="is")
    nc.vector.reciprocal(invs[:, 0:3], sums[:, 0:3])
    for j in range(3):
        nc.vector.tensor_scalar_mul(attn_t[j][:], attn_t[j][:], invs[:, j:j + 1])

    # ---------- fold attention into proj: M^T = A^T @ Wp^T ----------
    mts = [cst.tile([128, 384], dt.bfloat16, tag=f"mt{i}", name=f"mts{i}")
           for i in range(3)]
    for i in range(3):
        mtp = gtp.tile([128, 384], dt.float32, tag=f"g{i}", name=f"mtp{i}")
        for j in range(3):
            nc.tensor.matmul(
                mtp[:],
                lhsT=attn_t[j][:, i * 128:(i + 1) * 128],
                rhs=wp_sb[:, 384 * j:384 * (j + 1)],
                start=(j == 0), stop=(j == 2))
        nc.scalar.activation(mts[i][:], mtp[:], Act.Copy)

    # ---------- fused attn+proj: out = (Wp A) v ----------
    for ob in range(3):
        for t in range(16):
            hh, lr = (4 * t) // HR, (4 * t) % HR
            ps = mmp.tile([128, 512], dt.float32, tag="dw", bufs=3, name="pso")
            for i in range(3):
                vv = dv[(hh, i)][:].rearrange("p (r c) -> p r c", c=WP)
                nc.tensor.matmul(
                    ps[:],
                    lhsT=mts[i][:, ob * 128:(ob + 1) * 128],
                    rhs=vv[:, lr:lr + 4, 0:128],
                    start=(i == 0), stop=(i == 2))
            ot = wk.tile([128, 512], dt.bfloat16, tag="ob", bufs=3, name="ot")
            if t % 2 == 0:
                nc.scalar.activation(ot[:], ps[:], Act.Copy)
            else:
                nc.vector.tensor_copy(ot[:], ps[:])
            nc.gpsimd.dma_start(out=outd.ap()[ob, :, 512 * t:512 * (t + 1)],
                                in_=ot[:])

    for p in (drp, gtp, mmp, wk, dwp, tmpp, qp, xp, cst):
        p.release()


def build_nc():
    if "nc" in _CACHE:
        return _CACHE["nc"]
    from concourse import bacc, tile
    import concourse.mybir as mybir
    dt = mybir.dt
    nc = bacc.Bacc("TRN2", target_bir_lowering=False, debug=False, num_devices=8)
    xd = nc.dram_tensor("x", [3, 128, 66 * WP], dt.bfloat16, kind="ExternalInput")
    wqd = nc.dram_tensor("wq", [128, 9 * 384], dt.bfloat16, kind="ExternalInput")
    dtd = nc.dram_tensor("dtap", [128, 9 * 5 * 128], dt.bfloat16, kind="ExternalInput")
    wpd = nc.dram_tensor("wp", [128, 3 * 384], dt.bfloat16, kind="ExternalInput")
    idd = nc.dram_tensor("identb", [128, 128], dt.bfloat16, kind="ExternalInput")
    mkd = nc.dram_tensor("maskt", [128, 3 * 384], dt.float32, kind="ExternalInput")
    tpd = nc.dram_tensor("tempc", [128, 4], dt.float32, kind="ExternalInput")
    dsd = nc.dram_tensor("dwsc", [128, 81], dt.float32, kind="ExternalInput")
    x8d = nc.dram_tensor("x8", [3, 128, 66 * WP], dt.float8e4, kind="ExternalInput")
    w8d = nc.dram_tensor("wq8", [128, 6 * 384], dt.float8e4, kind="ExternalInput")
    outd = nc.dram_tensor("out", [3, 128, NT], dt.bfloat16, kind="ExternalOutput")
    with tile.TileContext(nc) as tc:
        _build_body(nc, tc, (xd, wqd, dtd, wpd, idd, mkd, tpd, dsd, outd,
                             x8d, w8d))
    nc.compile()
    _CACHE["nc"] = nc
    return nc


def make_in_maps(x, qkv_w, dw_w, proj_w, temperature):
    x = np.asarray(x, np.float32)
    qkv_w = np.asarray(qkv_w, np.float32)
    dw_w = np.asarray(dw_w, np.float32)
    proj_w = np.asarray(proj_w, np.float32)
    temperature = np.asarray(temperature, np.float32).reshape(-1)

    # frame layout: padded row r holds image row r-1; cols 0..127 = x, 128/129 = 0
    xpad = np.zeros((B, C, 130, 130), np.float32)
    xpad[:, :, 1:129, 0:128] = x

    wq = np.zeros((128, 9 * 384), np.float32)
    for ob in range(9):
        for ch in range(3):
            blk = qkv_w[ob * 128:(ob + 1) * 128, ch * 128:(ch + 1) * 128]
            wq[:, ob * 384 + ch * 128: ob * 384 + (ch + 1) * 128] = blk.T
    dtap = np.zeros((128, 9 * 5 * 128), np.float32)
    rng = np.arange(128)
    for ob in range(9):
        for ti, k9 in enumerate(PE_TAPS):
            col = (ob * 5 + ti) * 128
            dtap[rng, col + rng] = dw_w[ob * 128 + rng, 0, k9 // 3, k9 % 3]
    wpm = np.zeros((128, 3 * 384), np.float32)
    for ch in range(3):
        wpm[:, ch * 384:(ch + 1) * 384] = proj_w[:, ch * 128:(ch + 1) * 128].T
    ident = np.eye(128, dtype=np.float32)
    mask = np.full((128, 3 * 384), -1e30, np.float32)
    for j in range(3):
        for p in range(128):
            hgrp = (128 * j + p) // CHD
            mask[p, 384 * j + CHD * hgrp: 384 * j + CHD * (hgrp + 1)] = 0.0
    tempc = np.zeros((128, 4), np.float32)
    for j in range(3):
        for p in range(128):
            tempc[p, j] = temperature[(128 * j + p) // CHD]
    dwsc = np.zeros((128, 81), np.float32)
    for ob in range(9):
        for k9 in range(9):
            dwsc[:, ob * 9 + k9] = dw_w[ob * 128:(ob + 1) * 128, 0,
                                        k9 // 3, k9 % 3]

    F8 = ml_dtypes.float8_e4m3fn
    shared = {
        "wq": wq.astype(BF16), "dtap": dtap.astype(BF16),
        "wq8": (wq[:, :6 * 384] * 64.0).astype(F8),
        "wp": wpm.astype(BF16), "identb": ident.astype(BF16),
        "maskt": mask, "tempc": tempc, "dwsc": dwsc,
    }
    in_maps = []
    for core in range(8):
        b, s = core // 2, core % 2
        xs = xpad[b, :, 64 * s: 64 * s + 66, :]          # [384, 66, 130]
        xs = np.ascontiguousarray(xs.reshape(3, 128, 66 * WP))
        m = dict(shared)
        m["x"] = xs.astype(BF16)
        m["x8"] = xs.astype(F8)
        in_maps.append(m)
    return in_maps


def assemble(results):
    full = np.zeros((B, C, 128, 128), np.float32)
    for core in range(8):
        b, s = core // 2, core % 2
        o = np.asarray(results[core]["out"], np.float32).reshape(C, 64, 128)
        full[b, :, 64 * s: 64 * s + 64, :] = o
    return full


def kernel(x, qkv_w, dw_w, proj_w, temperature):
    from concourse.bass_utils import run_bass_kernel_spmd
    nc = build_nc()
    in_maps = make_in_maps(x, qkv_w, dw_w, proj_w, temperature)
    res = run_bass_kernel_spmd(nc, in_maps, core_ids=list(range(8)))
    return assemble(res.results)


# revision 15
# speedup vs baseline: 1.0374x; 1.0374x over previous
"""XCA (cross-covariance) attention block on 8 TRN2 NeuronCores.

Sharding: 8 cores = 4 batches x 2 image-row-halves. Each core computes the
full pipeline (1x1 qkv conv -> 3x3 depthwise conv -> per-head channel
attention -> 1x1 proj) for its (batch, row-half). Gram matrices + sq-norms
need global spatial sums -> pairwise all-reduce of a tiny f32 buffer.

v2: right-pad-only frame layout (even offsets -> DVE 2x STT taps), center
tap fused into the psum evacuation, tap-major PE matmul rounds, attention
folded into the projection weights (M = Wp @ A), balanced engine split.
"""

import numpy as np
import ml_dtypes

B, C = 4, 384
HEADS, CHD = 8, 48
WP = 130            # row pitch: cols 0..127 valid, 128/129 zero pad
HR = 32             # valid rows per half
XROWS = HR + 2      # qkv rows per half (halo)
SP_H = XROWS * WP   # 4420
DWH = HR * WP       # 4160 dw output window (rows 1..32, cols 0..129)
LEAD = 2            # lead zero cols in qkv_pre (for tap delta=-131)
QPW = LEAD + SP_H + 4
SP8 = 4432          # fp8 x row-pitch per channel block (16B aligned)
DWBASE = LEAD + WP  # qkv_pre index of output (row 1, x 0) = 132 (even)
NT = 8192
BF16 = ml_dtypes.bfloat16

PE_TAPS = (2, 3, 5, 6, 8)   # odd-offset taps on TensorE (diag matmuls)
SC_TAP = 0                  # (-1,-1): ScalarE scaled copy + DVE add
DVE_TAPS = (1, 7)           # (dy=-1/+1, dx=0): even offsets -> STT 2x
FUSED_TAP = 4               # center tap fused into psum->sbuf STT

_CACHE = {}


def _tiles(total, step=512):
    out = []
    s = 0
    while s < total:
        out.append((s, min(step, total - s)))
        s += step
    return out


def _delta(k9):
    return WP * (k9 // 3 - 1) + (k9 % 3 - 1)


def _build_body(nc, tc, tens):
    import concourse.mybir as mybir
    dt = mybir.dt
    Alu = mybir.AluOpType
    Act = mybir.ActivationFunctionType
    AX = mybir.AxisListType

    xd, wqd, dtd, wpd, idd, mkd, tpd, dsd, outd, x8d, w8d = tens

    cst = tc.alloc_tile_pool(name="cst", bufs=1)
    xp = tc.alloc_tile_pool(name="xp", bufs=1)
    qp = tc.alloc_tile_pool(name="qp", bufs=2)
    tmpp = tc.alloc_tile_pool(name="tmp", bufs=2)
    dwp = tc.alloc_tile_pool(name="dwp", bufs=1)
    wk = tc.alloc_tile_pool(name="wk", bufs=2)
    mmp = tc.alloc_tile_pool(name="mm", bufs=1, space="PSUM")
    gtp = tc.alloc_tile_pool(name="gt", bufs=1, space="PSUM")
    drp = tc.alloc_tile_pool(name="dr", bufs=1, space="DRAM")

    # ---- constants in ----
    wq = cst.tile([128, 9 * 384], dt.bfloat16, tag="wq")
    nc.sync.dma_start(out=wq[:], in_=wqd.ap())
    wp_sb = cst.tile([128, 3 * 384], dt.bfloat16, tag="wp")
    nc.sync.dma_start(out=wp_sb[:], in_=wpd.ap())
    identb = cst.tile([128, 128], dt.bfloat16, tag="id")
    nc.sync.dma_start(out=identb[:], in_=idd.ap())
    mask_sb = cst.tile([128, 3 * 384], dt.bfloat16, tag="mk")
    nc.sync.dma_start(out=mask_sb[:], in_=mkd.ap())
    tempc_sb = cst.tile([128, 4], dt.float32, tag="tc")
    nc.sync.dma_start(out=tempc_sb[:], in_=tpd.ap())
    dwsc_sb = cst.tile([128, 81], dt.float32, tag="ds")
    nc.sync.dma_start(out=dwsc_sb[:], in_=dsd.ap())
    wq8 = cst.tile([128, 6 * 384], dt.float8e4, tag="wq8")
    nc.sync.dma_start(out=wq8[:], in_=w8d.ap())

    nsum = cst.tile([128, 12], dt.float32, tag="ns")

    def load_x(h):
        xs = []
        for ch in range(3):
            t = xp.tile([128, SP_H], dt.bfloat16, tag=f"x{ch}", name=f"x{ch}")
            nc.sync.dma_start(
                out=t[:], in_=xd.ap()[ch, :, HR * h * WP: HR * h * WP + SP_H])
            xs.append(t)
        x8 = xp.tile([128, 3 * SP8], dt.float8e4, tag="x8", name="x8")
        for ch in range(3):
            nc.sync.dma_start(
                out=x8[:, ch * SP8: ch * SP8 + SP_H],
                in_=x8d.ap()[ch, :, HR * h * WP: HR * h * WP + SP_H])
        return xs, x8

    def conv_block(blk, xs, dwt, nslot, vtags=False, x8=None):
        """1x1 qkv conv + 3x3 depthwise for channel block blk -> dwt."""
        dtt = wk.tile([128, 5 * 128], dt.bfloat16, tag="dt", name="dtt")
        nc.sync.dma_start(out=dtt[:], in_=dtd.ap()[:, blk * 640:(blk + 1) * 640])
        qpre = qp.tile([128, QPW], dt.bfloat16, tag="qp", name="qpre")
        nc.vector.memset(qpre[:, 0:LEAD], 0.0)
        nc.vector.memset(qpre[:, LEAD + SP_H:QPW], 0.0)
        x8r = x8[:].rearrange("p (c n) -> p c n", c=3) if x8 is not None else None
        for (s, n) in _tiles(SP_H):
            ps = mmp.tile([128, 512], dt.float32, tag="qk", bufs=2, name="psq")
            if x8 is not None:
                w8r = wq8[:].rearrange("p (b c m) -> p b c m", b=6, m=128)
                nc.tensor.matmul(
                    ps[:, :n], lhsT=w8r[:, blk, 0:2, :],
                    rhs=x8r[:, 0:2, s:s + n], start=True, stop=False,
                    perf_mode=mybir.MatmulPerfMode.DoubleRow)
                nc.tensor.matmul(
                    ps[:, :n], lhsT=w8r[:, blk, 2, :],
                    rhs=x8r[:, 2, s:s + n], start=False, stop=True)
                nc.scalar.activation(qpre[:, LEAD + s: LEAD + s + n],
                                     ps[:, :n], Act.Copy, scale=1.0 / 64.0)
            else:
                for ch in range(3):
                    nc.tensor.matmul(
                        ps[:, :n],
                        lhsT=wq[:, blk * 384 + ch * 128: blk * 384 + (ch + 1) * 128],
                        rhs=xs[ch][:, s:s + n],
                        start=(ch == 0), stop=(ch == 2))
                nc.scalar.activation(qpre[:, LEAD + s: LEAD + s + n],
                                     ps[:, :n], Act.Copy)
        # PE taps (tap-major within each 512 chunk) + fused evac/center-tap
        for ci, (s, n) in enumerate(_tiles(DWH)):
            if vtags:
                ps = gtp.tile([128, 512], dt.float32, tag=f"g{ci % 3}",
                              name="psd")
            else:
                ps = mmp.tile([128, 512], dt.float32, tag="dw", bufs=3,
                              name="psd")
            for ti, k9 in enumerate(PE_TAPS):
                d = _delta(k9)
                nc.tensor.matmul(
                    ps[:, :n],
                    lhsT=dtt[:, ti * 128:(ti + 1) * 128],
                    rhs=qpre[:, DWBASE + s + d: DWBASE + s + d + n],
                    start=(ti == 0), stop=(ti == len(PE_TAPS) - 1))
            nc.vector.scalar_tensor_tensor(
                dwt[:, s:s + n], qpre[:, DWBASE + s: DWBASE + s + n],
                dwsc_sb[:, blk * 9 + FUSED_TAP: blk * 9 + FUSED_TAP + 1],
                ps[:, :n], op0=Alu.mult, op1=Alu.add)
        for k9 in DVE_TAPS:
            d = _delta(k9)
            tt = tmpp.tile([128, DWH], dt.bfloat16, tag="tt", name="tt",
                           bufs=1)
            nc.vector.tensor_scalar_mul(
                tt[:], qpre[:, DWBASE + d: DWBASE + d + DWH],
                dwsc_sb[:, blk * 9 + k9: blk * 9 + k9 + 1])
            nc.vector.tensor_add(dwt[:], dwt[:], tt[:])
        tm = tmpp.tile([128, DWH], dt.bfloat16, tag="tm", name="tm", bufs=1)
        nc.scalar.activation(
            tm[:], qpre[:, DWBASE - 131: DWBASE - 131 + DWH], Act.Copy,
            scale=dwsc_sb[:, blk * 9 + SC_TAP: blk * 9 + SC_TAP + 1])
        nc.vector.tensor_add(dwt[:], dwt[:], tm[:])
        if nslot is not None:
            no = tmpp.tile([128, HR * 128], dt.bfloat16, tag="tm", name="no",
                           bufs=1)
            nc.scalar.activation(
                no[:].rearrange("p (r c) -> p r c", r=HR),
                dwt[:].rearrange("p (r c) -> p r c", c=WP)[:, :, 0:128],
                Act.Square, accum_out=nsum[:, nslot:nslot + 1])

    def phase_b(h, dq, dk, gt):
        # transposes + gram for half h; psum from the "dw" tag only, so the
        # next half's qkv matmuls ("qk" tag) can overlap.
        for j in range(HR):
            qtp = mmp.tile([128, 384], dt.bfloat16, tag="dw", bufs=3, name="qtp")
            ktp = mmp.tile([128, 384], dt.bfloat16, tag="dw", bufs=3, name="ktp")
            for b in range(3):
                nc.tensor.transpose(
                    qtp[:, b * 128:(b + 1) * 128],
                    dq[b][:, j * WP: j * WP + 128].opt(), identb[:])
                nc.tensor.transpose(
                    ktp[:, b * 128:(b + 1) * 128],
                    dk[b][:, j * WP: j * WP + 128].opt(), identb[:])
            qts = wk.tile([128, 768], dt.bfloat16, tag="qts", bufs=3, name="qts")
            if j % 2 == 0:
                nc.scalar.activation(qts[:, 0:384], qtp[:], Act.Copy)
                nc.vector.tensor_copy(qts[:, 384:768], ktp[:])
            else:
                nc.vector.tensor_copy(qts[:, 0:384], qtp[:])
                nc.scalar.activation(qts[:, 384:768], ktp[:], Act.Copy)
            for i in range(3):
                nc.tensor.matmul(
                    gt[i][:],
                    lhsT=qts[:, 384 + i * 128: 384 + (i + 1) * 128],
                    rhs=qts[:, 0:384],
                    start=(j == 0), stop=(j == HR - 1))

    # ---------- main: per half: q,k blocks -> phase B -> collective, v ----------
    dv = {}
    g_half = []
    for h in range(2):
        xs, x8 = load_x(h)
        dq = [dwp.tile([128, DWH], dt.bfloat16, tag=f"d{i}", name=f"dq{i}")
              for i in range(3)]
        dk = [dwp.tile([128, DWH], dt.bfloat16, tag=f"d{3 + i}", name=f"dk{i}")
              for i in range(3)]
        for i in range(3):
            conv_block(i, xs, dq[i], h * 6 + i, x8=x8)
            conv_block(3 + i, xs, dk[i], h * 6 + 3 + i, x8=x8)
        gt = [gtp.tile([128, 384], dt.float32, tag=f"g{i}", name=f"gt{i}")
              for i in range(3)]
        phase_b(h, dq, dk, gt)
        # per-half pairwise all-reduce of gram + sq-norms (overlaps with the
        # v blocks / next half)
        cc_sb = cst.tile([128, 1158], dt.float32, tag=f"cc{h}", name="cc_sb")
        for i in range(3):
            nc.vector.tensor_copy(cc_sb[:, 384 * i:384 * (i + 1)], gt[i][:])
        nc.vector.tensor_copy(cc_sb[:, 1152:1158], nsum[:, h * 6:h * 6 + 6])
        cc_in = drp.tile([128, 1158], dt.float32, tag=f"ccin{h}", name="cc_in")
        cc_out = drp.tile([128, 1158], dt.float32, tag=f"ccout{h}", name="cc_out")
        nc.gpsimd.dma_start(out=cc_in[:], in_=cc_sb[:])
        nc.gpsimd.collective_compute(
            "AllReduce", Alu.add,
            replica_groups=[[0, 1], [2, 3], [4, 5], [6, 7]],
            ins=[cc_in.opt()], outs=[cc_out.opt()])
        gh = cst.tile([128, 1158], dt.float32, tag=f"gh{h}", name="g_half")
        nc.gpsimd.dma_start(out=gh[:], in_=cc_out[:])
        g_half.append(gh)
        # v blocks for this half (h0 -> fresh tags d6-d8; h1 -> reuse d0-d2,
        # whose q tiles are dead after phase_b(h1))
        for i in range(3):
            t = dwp.tile([128, DWH], dt.bfloat16,
                         tag=(f"d{6 + i}" if h == 0 else f"d{i}"),
                         name=f"dv{i}_{h}")
            conv_block(6 + i, xs, t, None, vtags=True)
            dv[(h, i)] = t

    g_sb = g_half[0]
    nc.vector.tensor_add(g_sb[:], g_half[0][:], g_half[1][:])

    # ---------- softmax (tiny) ----------
    rt = cst.tile([128, 8], dt.float32, tag="rt")
    nc.scalar.activation(rt[:, 0:6], g_sb[:, 1152:1158], Act.Sqrt)
    nc.vector.tensor_scalar_max(rt[:, 0:6], rt[:, 0:6], 1e-12)
    inv = cst.tile([128, 8], dt.float32, tag="inv")
    nc.vector.reciprocal(inv[:, 0:6], rt[:, 0:6])

    gts_t = [cst.tile([128, 384], dt.bfloat16, tag=f"gs{i}", name=f"gsb{i}")
             for i in range(3)]
    for i in range(3):  # row-scale G^T (rows = k channels, block i) by inv_k
        nc.vector.tensor_scalar_mul(
            gts_t[i][:], g_sb[:, 384 * i:384 * (i + 1)], inv[:, 3 + i:4 + i])
    gs_ps = [gtp.tile([128, 384], dt.bfloat16, tag=f"g{j}", name=f"gsp{j}")
             for j in range(3)]
    for j in range(3):
        for i in range(3):
            nc.tensor.transpose(
                gs_ps[j][:, i * 128:(i + 1) * 128],
                gts_t[i][:, j * 128:(j + 1) * 128], identb[:])
    attn_t = [cst.tile([128, 384], dt.bfloat16, tag=f"at{j}", name=f"attn{j}")
              for j in range(3)]
    sums = cst.tile([128, 4], dt.float32, tag="sm")
    for j in range(3):
        zf = wk.tile([128, 384], dt.float32, tag="zf", name="zf")
        nc.vector.tensor_scalar_mul(zf[:], gs_ps[j][:], inv[:, j:j + 1])
        z = wk.tile([128, 384], dt.float32, tag="zz", name="zz")
        nc.vector.scalar_tensor_tensor(
            z[:], zf[:], tempc_sb[:, j:j + 1],
            mask_sb[:, 384 * j:384 * (j + 1)],
            op0=Alu.mult, op1=Alu.add)
        negmax = wk.tile([128, 1], dt.float32, tag="nm", name="nm")
        nc.vector.tensor_reduce(negmax[:], z[:], AX.X, Alu.max, negate=True)
        nc.scalar.activation(
            attn_t[j][:], z[:], Act.Exp, bias=negmax[:, 0:1],
            accum_out=sums[:, j:j + 1])
    invs = cst.tile([128, 4], dt.float32, tag="is")
    nc.vector.reciprocal(invs[:, 0:3], sums[:, 0:3])
    for j in range(3):
        nc.vector.tensor_scalar_mul(attn_t[j][:], attn_t[j][:], invs[:, j:j + 1])

    # ---------- fold attention into proj: M^T = A^T @ Wp^T ----------
    mts = [cst.tile([128, 384], dt.bfloat16, tag=f"mt{i}", name=f"mts{i}")
           for i in range(3)]
    for i in range(3):
        mtp = gtp.tile([128, 384], dt.float32, tag=f"g{i}", name=f"mtp{i}")
        for j in range(3):
            nc.tensor.matmul(
                mtp[:],
                lhsT=attn_t[j][:, i * 128:(i + 1) * 128],
                rhs=wp_sb[:, 384 * j:384 * (j + 1)],
                start=(j == 0), stop=(j == 2))
        nc.scalar.activation(mts[i][:], mtp[:], Act.Copy)

    # ---------- fused attn+proj: out = (Wp A) v ----------
    for ob in range(3):
        for t in range(16):
            hh, lr = (4 * t) // HR, (4 * t) % HR
            ps = mmp.tile([128, 512], dt.float32, tag="dw", bufs=3, name="pso")
            for i in range(3):
                vv = dv[(hh, i)][:].rearrange("p (r c) -> p r c", c=WP)
                nc.tensor.matmul(
                    ps[:],
                    lhsT=mts[i][:, ob * 128:(ob + 1) * 128],
                    rhs=vv[:, lr:lr + 4, 0:128],
                    start=(i == 0), stop=(i == 2))
            ot = wk.tile([128, 512], dt.bfloat16, tag="ob", bufs=3, name="ot")
            if t % 2 == 0:
                nc.scalar.activation(ot[:], ps[:], Act.Copy)
            else:
                nc.vector.tensor_copy(ot[:], ps[:])
            nc.gpsimd.dma_start(out=outd.ap()[ob, :, 512 * t:512 * (t + 1)],
                                in_=ot[:])

    for p in (drp, gtp, mmp, wk, dwp, tmpp, qp, xp, cst):
        p.release()


def build_nc():
    if "nc" in _CACHE:
        return _CACHE["nc"]
    from concourse import bacc, tile
    import concourse.mybir as mybir
    dt = mybir.dt
    nc = bacc.Bacc("TRN2", target_bir_lowering=False, debug=False, num_devices=8)
    xd = nc.dram_tensor("x", [3, 128, 66 * WP], dt.bfloat16, kind="ExternalInput")
    wqd = nc.dram_tensor("wq", [128, 9 * 384], dt.bfloat16, kind="ExternalInput")
    dtd = nc.dram_tensor("dtap", [128, 9 * 5 * 128], dt.bfloat16, kind="ExternalInput")
    wpd = nc.dram_tensor("wp", [128, 3 * 384], dt.bfloat16, kind="ExternalInput")
    idd = nc.dram_tensor("identb", [128, 128], dt.bfloat16, kind="ExternalInput")
    mkd = nc.dram_tensor("maskt", [128, 3 * 384], dt.bfloat16, kind="ExternalInput")
    tpd = nc.dram_tensor("tempc", [128, 4], dt.float32, kind="ExternalInput")
    dsd = nc.dram_tensor("dwsc", [128, 81], dt.float32, kind="ExternalInput")
    x8d = nc.dram_tensor("x8", [3, 128, 66 * WP], dt.float8e4, kind="ExternalInput")
    w8d = nc.dram_tensor("wq8", [128, 6 * 384], dt.float8e4, kind="ExternalInput")
    outd = nc.dram_tensor("out", [3, 128, NT], dt.bfloat16, kind="ExternalOutput")
    with tile.TileContext(nc) as tc:
        _build_body(nc, tc, (xd, wqd, dtd, wpd, idd, mkd, tpd, dsd, outd,
                             x8d, w8d))
    nc.compile()
    _CACHE["nc"] = nc
    return nc


def make_in_maps(x, qkv_w, dw_w, proj_w, temperature):
    x = np.asarray(x, np.float32)
    qkv_w = np.asarray(qkv_w, np.float32)
    dw_w = np.asarray(dw_w, np.float32)
    proj_w = np.asarray(proj_w, np.float32)
    temperature = np.asarray(temperature, np.float32).reshape(-1)

    # frame layout: padded row r holds image row r-1; cols 0..127 = x, 128/129 = 0
    xpad = np.zeros((B, C, 130, 130), np.float32)
    xpad[:, :, 1:129, 0:128] = x

    wq = np.zeros((128, 9 * 384), np.float32)
    for ob in range(9):
        for ch in range(3):
            blk = qkv_w[ob * 128:(ob + 1) * 128, ch * 128:(ch + 1) * 128]
            wq[:, ob * 384 + ch * 128: ob * 384 + (ch + 1) * 128] = blk.T
    dtap = np.zeros((128, 9 * 5 * 128), np.float32)
    rng = np.arange(128)
    for ob in range(9):
        for ti, k9 in enumerate(PE_TAPS):
            col = (ob * 5 + ti) * 128
            dtap[rng, col + rng] = dw_w[ob * 128 + rng, 0, k9 // 3, k9 % 3]
    wpm = np.zeros((128, 3 * 384), np.float32)
    for ch in range(3):
        wpm[:, ch * 384:(ch + 1) * 384] = proj_w[:, ch * 128:(ch + 1) * 128].T
    ident = np.eye(128, dtype=np.float32)
    mask = np.full((128, 3 * 384), -1e30, np.float32)
    for j in range(3):
        for p in range(128):
            hgrp = (128 * j + p) // CHD
            mask[p, 384 * j + CHD * hgrp: 384 * j + CHD * (hgrp + 1)] = 0.0
    tempc = np.zeros((128, 4), np.float32)
    for j in range(3):
        for p in range(128):
            tempc[p, j] = temperature[(128 * j + p) // CHD]
    dwsc = np.zeros((128, 81), np.float32)
    for ob in range(9):
        for k9 in range(9):
            dwsc[:, ob * 9 + k9] = dw_w[ob * 128:(ob + 1) * 128, 0,
                                        k9 // 3, k9 % 3]

    F8 = ml_dtypes.float8_e4m3fn
    shared = {
        "wq": wq.astype(BF16), "dtap": dtap.astype(BF16),
        "wq8": (wq[:, :6 * 384] * 64.0).astype(F8),
        "wp": wpm.astype(BF16), "identb": ident.astype(BF16),
        "maskt": mask.astype(BF16), "tempc": tempc, "dwsc": dwsc,
    }
    in_maps = []
    for core in range(8):
        b, s = core // 2, core % 2
        xs = xpad[b, :, 64 * s: 64 * s + 66, :]          # [384, 66, 130]
        xs = np.ascontiguousarray(xs.reshape(3, 128, 66 * WP))
        m = dict(shared)
        m["x"] = xs.astype(BF16)
        m["x8"] = xs.astype(F8)
        in_maps.append(m)
    return in_maps


def assemble(results):
    full = np.zeros((B, C, 128, 128), np.float32)
    for core in range(8):
        b, s = core // 2, core % 2
        o = np.asarray(results[core]["out"], np.float32).reshape(C, 64, 128)
        full[b, :, 64 * s: 64 * s + 64, :] = o
    return full


def kernel(x, qkv_w, dw_w, proj_w, temperature):
    from concourse.bass_utils import run_bass_kernel_spmd
    nc = build_nc()
    in_maps = make_in_maps(x, qkv_w, dw_w, proj_w, temperature)
    res = run_bass_kernel_spmd(nc, in_maps, core_ids=list(range(8)))
    return assemble(res.results)


# revision 16
# speedup vs baseline: 1.0572x; 1.0191x over previous
"""XCA (cross-covariance) attention block on 8 TRN2 NeuronCores.

Sharding: 8 cores = 4 batches x 2 image-row-halves. Each core computes the
full pipeline (1x1 qkv conv -> 3x3 depthwise conv -> per-head channel
attention -> 1x1 proj) for its (batch, row-half). Gram matrices + sq-norms
need global spatial sums -> pairwise all-reduce of a tiny f32 buffer.

v2: right-pad-only frame layout (even offsets -> DVE 2x STT taps), center
tap fused into the psum evacuation, tap-major PE matmul rounds, attention
folded into the projection weights (M = Wp @ A), balanced engine split.
"""

import numpy as np
import ml_dtypes

B, C = 4, 384
HEADS, CHD = 8, 48
WP = 130            # row pitch: cols 0..127 valid, 128/129 zero pad
HR = 32             # valid rows per half
XROWS = HR + 2      # qkv rows per half (halo)
SP_H = XROWS * WP   # 4420
DWH = HR * WP       # 4160 dw output window (rows 1..32, cols 0..129)
LEAD = 2            # lead zero cols in qkv_pre (for tap delta=-131)
QPW = LEAD + SP_H + 4
SP8 = 4432          # fp8 x row-pitch per channel block (16B aligned)
DWBASE = LEAD + WP  # qkv_pre index of output (row 1, x 0) = 132 (even)
NT = 8192
BF16 = ml_dtypes.bfloat16

PE_TAPS = (2, 3, 5, 6, 8, 7)   # diag-matmul taps on TensorE
SC_TAP = 0                  # (-1,-1): ScalarE scaled copy + DVE add
DVE_TAPS = (1,)             # (dy=-1, dx=0): DVE TS(4x) + TT(2x)
FUSED_TAP = 4               # center tap fused into psum->sbuf STT

_CACHE = {}


def _tiles(total, step=512):
    out = []
    s = 0
    while s < total:
        out.append((s, min(step, total - s)))
        s += step
    return out


def _delta(k9):
    return WP * (k9 // 3 - 1) + (k9 % 3 - 1)


def _build_body(nc, tc, tens):
    import concourse.mybir as mybir
    dt = mybir.dt
    Alu = mybir.AluOpType
    Act = mybir.ActivationFunctionType
    AX = mybir.AxisListType

    xd, wqd, dtd, wpd, idd, mkd, tpd, dsd, outd, x8d, w8d = tens

    cst = tc.alloc_tile_pool(name="cst", bufs=1)
    xp = tc.alloc_tile_pool(name="xp", bufs=1)
    qp = tc.alloc_tile_pool(name="qp", bufs=2)
    tmpp = tc.alloc_tile_pool(name="tmp", bufs=2)
    dwp = tc.alloc_tile_pool(name="dwp", bufs=1)
    wk = tc.alloc_tile_pool(name="wk", bufs=2)
    mmp = tc.alloc_tile_pool(name="mm", bufs=1, space="PSUM")
    gtp = tc.alloc_tile_pool(name="gt", bufs=1, space="PSUM")
    drp = tc.alloc_tile_pool(name="dr", bufs=1, space="DRAM")

    # ---- constants in ----
    wq = cst.tile([128, 9 * 384], dt.bfloat16, tag="wq")
    nc.sync.dma_start(out=wq[:], in_=wqd.ap())
    wp_sb = cst.tile([128, 3 * 384], dt.bfloat16, tag="wp")
    nc.sync.dma_start(out=wp_sb[:], in_=wpd.ap())
    identb = cst.tile([128, 128], dt.bfloat16, tag="id")
    nc.sync.dma_start(out=identb[:], in_=idd.ap())
    mask_sb = cst.tile([128, 3 * 384], dt.bfloat16, tag="mk")
    nc.sync.dma_start(out=mask_sb[:], in_=mkd.ap())
    tempc_sb = cst.tile([128, 4], dt.float32, tag="tc")
    nc.sync.dma_start(out=tempc_sb[:], in_=tpd.ap())
    dwsc_sb = cst.tile([128, 81], dt.float32, tag="ds")
    nc.sync.dma_start(out=dwsc_sb[:], in_=dsd.ap())
    wq8 = cst.tile([128, 6 * 384], dt.float8e4, tag="wq8")
    nc.sync.dma_start(out=wq8[:], in_=w8d.ap())

    nsum = cst.tile([128, 12], dt.float32, tag="ns")

    def load_x(h):
        xs = []
        for ch in range(3):
            t = xp.tile([128, SP_H], dt.bfloat16, tag=f"x{ch}", name=f"x{ch}")
            nc.sync.dma_start(
                out=t[:], in_=xd.ap()[ch, :, HR * h * WP: HR * h * WP + SP_H])
            xs.append(t)
        x8 = xp.tile([128, 3 * SP8], dt.float8e4, tag="x8", name="x8")
        for ch in range(3):
            nc.sync.dma_start(
                out=x8[:, ch * SP8: ch * SP8 + SP_H],
                in_=x8d.ap()[ch, :, HR * h * WP: HR * h * WP + SP_H])
        return xs, x8

    def conv_block(blk, xs, dwt, nslot, vtags=False, x8=None):
        """1x1 qkv conv + 3x3 depthwise for channel block blk -> dwt."""
        dtt = wk.tile([128, 6 * 128], dt.bfloat16, tag="dt", name="dtt")
        nc.sync.dma_start(out=dtt[:], in_=dtd.ap()[:, blk * 768:(blk + 1) * 768])
        qpre = qp.tile([128, QPW], dt.bfloat16, tag="qp", name="qpre")
        nc.vector.memset(qpre[:, 0:LEAD], 0.0)
        nc.vector.memset(qpre[:, LEAD + SP_H:QPW], 0.0)
        x8r = x8[:].rearrange("p (c n) -> p c n", c=3) if x8 is not None else None
        for (s, n) in _tiles(SP_H):
            ps = mmp.tile([128, 512], dt.float32, tag="qk", bufs=2, name="psq")
            if x8 is not None:
                w8r = wq8[:].rearrange("p (b c m) -> p b c m", b=6, m=128)
                nc.tensor.matmul(
                    ps[:, :n], lhsT=w8r[:, blk, 0:2, :],
                    rhs=x8r[:, 0:2, s:s + n], start=True, stop=False,
                    perf_mode=mybir.MatmulPerfMode.DoubleRow)
                nc.tensor.matmul(
                    ps[:, :n], lhsT=w8r[:, blk, 2, :],
                    rhs=x8r[:, 2, s:s + n], start=False, stop=True)
                nc.scalar.activation(qpre[:, LEAD + s: LEAD + s + n],
                                     ps[:, :n], Act.Copy, scale=1.0 / 64.0)
            else:
                for ch in range(3):
                    nc.tensor.matmul(
                        ps[:, :n],
                        lhsT=wq[:, blk * 384 + ch * 128: blk * 384 + (ch + 1) * 128],
                        rhs=xs[ch][:, s:s + n],
                        start=(ch == 0), stop=(ch == 2))
                nc.scalar.activation(qpre[:, LEAD + s: LEAD + s + n],
                                     ps[:, :n], Act.Copy)
        # PE taps (tap-major within each 512 chunk) + fused evac/center-tap
        for ci, (s, n) in enumerate(_tiles(DWH)):
            if vtags:
                ps = gtp.tile([128, 512], dt.float32, tag=f"g{ci % 3}",
                              name="psd")
            else:
                ps = mmp.tile([128, 512], dt.float32, tag="dw", bufs=3,
                              name="psd")
            for ti, k9 in enumerate(PE_TAPS):
                d = _delta(k9)
                nc.tensor.matmul(
                    ps[:, :n],
                    lhsT=dtt[:, ti * 128:(ti + 1) * 128],
                    rhs=qpre[:, DWBASE + s + d: DWBASE + s + d + n],
                    start=(ti == 0), stop=(ti == len(PE_TAPS) - 1))
            nc.vector.scalar_tensor_tensor(
                dwt[:, s:s + n], qpre[:, DWBASE + s: DWBASE + s + n],
                dwsc_sb[:, blk * 9 + FUSED_TAP: blk * 9 + FUSED_TAP + 1],
                ps[:, :n], op0=Alu.mult, op1=Alu.add)
        for k9 in DVE_TAPS:
            d = _delta(k9)
            tt = tmpp.tile([128, DWH], dt.bfloat16, tag="tt", name="tt",
                           bufs=1)
            nc.vector.tensor_scalar_mul(
                tt[:], qpre[:, DWBASE + d: DWBASE + d + DWH],
                dwsc_sb[:, blk * 9 + k9: blk * 9 + k9 + 1])
            nc.vector.tensor_add(dwt[:], dwt[:], tt[:])
        tm = tmpp.tile([128, DWH], dt.bfloat16, tag="tm", name="tm", bufs=1)
        nc.scalar.activation(
            tm[:], qpre[:, DWBASE - 131: DWBASE - 131 + DWH], Act.Copy,
            scale=dwsc_sb[:, blk * 9 + SC_TAP: blk * 9 + SC_TAP + 1])
        nc.vector.tensor_add(dwt[:], dwt[:], tm[:])
        if nslot is not None:
            no = tmpp.tile([128, HR * 128], dt.bfloat16, tag="tm", name="no",
                           bufs=1)
            nc.scalar.activation(
                no[:].rearrange("p (r c) -> p r c", r=HR),
                dwt[:].rearrange("p (r c) -> p r c", c=WP)[:, :, 0:128],
                Act.Square, accum_out=nsum[:, nslot:nslot + 1])

    def phase_b(h, dq, dk, gt):
        # transposes + gram for half h; psum from the "dw" tag only, so the
        # next half's qkv matmuls ("qk" tag) can overlap.
        for j in range(HR):
            qtp = mmp.tile([128, 384], dt.bfloat16, tag="dw", bufs=3, name="qtp")
            ktp = mmp.tile([128, 384], dt.bfloat16, tag="dw", bufs=3, name="ktp")
            for b in range(3):
                nc.tensor.transpose(
                    qtp[:, b * 128:(b + 1) * 128],
                    dq[b][:, j * WP: j * WP + 128].opt(), identb[:])
                nc.tensor.transpose(
                    ktp[:, b * 128:(b + 1) * 128],
                    dk[b][:, j * WP: j * WP + 128].opt(), identb[:])
            qts = wk.tile([128, 768], dt.bfloat16, tag="qts", bufs=3, name="qts")
            if j % 2 == 0:
                nc.scalar.activation(qts[:, 0:384], qtp[:], Act.Copy)
                nc.vector.tensor_copy(qts[:, 384:768], ktp[:])
            else:
                nc.vector.tensor_copy(qts[:, 0:384], qtp[:])
                nc.scalar.activation(qts[:, 384:768], ktp[:], Act.Copy)
            for i in range(3):
                nc.tensor.matmul(
                    gt[i][:],
                    lhsT=qts[:, 384 + i * 128: 384 + (i + 1) * 128],
                    rhs=qts[:, 0:384],
                    start=(j == 0), stop=(j == HR - 1))

    # ---------- main: per half: q,k blocks -> phase B -> collective, v ----------
    dv = {}
    g_half = []
    for h in range(2):
        xs, x8 = load_x(h)
        dq = [dwp.tile([128, DWH], dt.bfloat16, tag=f"d{i}", name=f"dq{i}")
              for i in range(3)]
        dk = [dwp.tile([128, DWH], dt.bfloat16, tag=f"d{3 + i}", name=f"dk{i}")
              for i in range(3)]
        for i in range(3):
            conv_block(i, xs, dq[i], h * 6 + i, x8=x8)
            conv_block(3 + i, xs, dk[i], h * 6 + 3 + i, x8=x8)
        gt = [gtp.tile([128, 384], dt.float32, tag=f"g{i}", name=f"gt{i}")
              for i in range(3)]
        phase_b(h, dq, dk, gt)
        # per-half pairwise all-reduce of gram + sq-norms (overlaps with the
        # v blocks / next half)
        cc_sb = cst.tile([128, 1158], dt.float32, tag=f"cc{h}", name="cc_sb")
        for i in range(3):
            nc.vector.tensor_copy(cc_sb[:, 384 * i:384 * (i + 1)], gt[i][:])
        nc.vector.tensor_copy(cc_sb[:, 1152:1158], nsum[:, h * 6:h * 6 + 6])
        cc_in = drp.tile([128, 1158], dt.float32, tag=f"ccin{h}", name="cc_in")
        cc_out = drp.tile([128, 1158], dt.float32, tag=f"ccout{h}", name="cc_out")
        nc.gpsimd.dma_start(out=cc_in[:], in_=cc_sb[:])
        nc.gpsimd.collective_compute(
            "AllReduce", Alu.add,
            replica_groups=[[0, 1], [2, 3], [4, 5], [6, 7]],
            ins=[cc_in.opt()], outs=[cc_out.opt()])
        gh = cst.tile([128, 1158], dt.float32, tag=f"gh{h}", name="g_half")
        nc.gpsimd.dma_start(out=gh[:], in_=cc_out[:])
        g_half.append(gh)
        # v blocks for this half (h0 -> fresh tags d6-d8; h1 -> reuse d0-d2,
        # whose q tiles are dead after phase_b(h1))
        for i in range(3):
            t = dwp.tile([128, DWH], dt.bfloat16,
                         tag=(f"d{6 + i}" if h == 0 else f"d{i}"),
                         name=f"dv{i}_{h}")
            conv_block(6 + i, xs, t, None, vtags=True)
            dv[(h, i)] = t

    g_sb = g_half[0]
    nc.vector.tensor_add(g_sb[:], g_half[0][:], g_half[1][:])

    # ---------- softmax (tiny) ----------
    rt = cst.tile([128, 8], dt.float32, tag="rt")
    nc.scalar.activation(rt[:, 0:6], g_sb[:, 1152:1158], Act.Sqrt)
    nc.vector.tensor_scalar_max(rt[:, 0:6], rt[:, 0:6], 1e-12)
    inv = cst.tile([128, 8], dt.float32, tag="inv")
    nc.vector.reciprocal(inv[:, 0:6], rt[:, 0:6])

    gts_t = [cst.tile([128, 384], dt.bfloat16, tag=f"gs{i}", name=f"gsb{i}")
             for i in range(3)]
    for i in range(3):  # row-scale G^T (rows = k channels, block i) by inv_k
        nc.vector.tensor_scalar_mul(
            gts_t[i][:], g_sb[:, 384 * i:384 * (i + 1)], inv[:, 3 + i:4 + i])
    gs_ps = [gtp.tile([128, 384], dt.bfloat16, tag=f"g{j}", name=f"gsp{j}")
             for j in range(3)]
    for j in range(3):
        for i in range(3):
            nc.tensor.transpose(
                gs_ps[j][:, i * 128:(i + 1) * 128],
                gts_t[i][:, j * 128:(j + 1) * 128], identb[:])
    attn_t = [cst.tile([128, 384], dt.bfloat16, tag=f"at{j}", name=f"attn{j}")
              for j in range(3)]
    sums = cst.tile([128, 4], dt.float32, tag="sm")
    for j in range(3):
        zf = wk.tile([128, 384], dt.float32, tag="zf", name="zf")
        nc.vector.tensor_scalar_mul(zf[:], gs_ps[j][:], inv[:, j:j + 1])
        z = wk.tile([128, 384], dt.float32, tag="zz", name="zz")
        nc.vector.scalar_tensor_tensor(
            z[:], zf[:], tempc_sb[:, j:j + 1],
            mask_sb[:, 384 * j:384 * (j + 1)],
            op0=Alu.mult, op1=Alu.add)
        negmax = wk.tile([128, 1], dt.float32, tag="nm", name="nm")
        nc.vector.tensor_reduce(negmax[:], z[:], AX.X, Alu.max, negate=True)
        nc.scalar.activation(
            attn_t[j][:], z[:], Act.Exp, bias=negmax[:, 0:1],
            accum_out=sums[:, j:j + 1])
    invs = cst.tile([128, 4], dt.float32, tag="is")
    nc.vector.reciprocal(invs[:, 0:3], sums[:, 0:3])
    for j in range(3):
        nc.vector.tensor_scalar_mul(attn_t[j][:], attn_t[j][:], invs[:, j:j + 1])

    # ---------- fold attention into proj: M^T = A^T @ Wp^T ----------
    mts = [cst.tile([128, 384], dt.bfloat16, tag=f"mt{i}", name=f"mts{i}")
           for i in range(3)]
    for i in range(3):
        mtp = gtp.tile([128, 384], dt.float32, tag=f"g{i}", name=f"mtp{i}")
        for j in range(3):
            nc.tensor.matmul(
                mtp[:],
                lhsT=attn_t[j][:, i * 128:(i + 1) * 128],
                rhs=wp_sb[:, 384 * j:384 * (j + 1)],
                start=(j == 0), stop=(j == 2))
        nc.scalar.activation(mts[i][:], mtp[:], Act.Copy)

    # ---------- fused attn+proj: out = (Wp A) v ----------
    for ob in range(3):
        for t in range(16):
            hh, lr = (4 * t) // HR, (4 * t) % HR
            ps = mmp.tile([128, 512], dt.float32, tag="dw", bufs=3, name="pso")
            for i in range(3):
                vv = dv[(hh, i)][:].rearrange("p (r c) -> p r c", c=WP)
                nc.tensor.matmul(
                    ps[:],
                    lhsT=mts[i][:, ob * 128:(ob + 1) * 128],
                    rhs=vv[:, lr:lr + 4, 0:128],
                    start=(i == 0), stop=(i == 2))
            ot = wk.tile([128, 512], dt.bfloat16, tag="ob", bufs=3, name="ot")
            if t % 2 == 0:
                nc.scalar.activation(ot[:], ps[:], Act.Copy)
            else:
                nc.vector.tensor_copy(ot[:], ps[:])
            nc.gpsimd.dma_start(out=outd.ap()[ob, :, 512 * t:512 * (t + 1)],
                                in_=ot[:])

    for p in (drp, gtp, mmp, wk, dwp, tmpp, qp, xp, cst):
        p.release()


def build_nc():
    if "nc" in _CACHE:
        return _CACHE["nc"]
    from concourse import bacc, tile
    import concourse.mybir as mybir
    dt = mybir.dt
    nc = bacc.Bacc("TRN2", target_bir_lowering=False, debug=False, num_devices=8)
    xd = nc.dram_tensor("x", [3, 128, 66 * WP], dt.bfloat16, kind="ExternalInput")
    wqd = nc.dram_tensor("wq", [128, 9 * 384], dt.bfloat16, kind="ExternalInput")
    dtd = nc.dram_tensor("dtap", [128, 9 * 6 * 128], dt.bfloat16, kind="ExternalInput")
    wpd = nc.dram_tensor("wp", [128, 3 * 384], dt.bfloat16, kind="ExternalInput")
    idd = nc.dram_tensor("identb", [128, 128], dt.bfloat16, kind="ExternalInput")
    mkd = nc.dram_tensor("maskt", [128, 3 * 384], dt.bfloat16, kind="ExternalInput")
    tpd = nc.dram_tensor("tempc", [128, 4], dt.float32, kind="ExternalInput")
    dsd = nc.dram_tensor("dwsc", [128, 81], dt.float32, kind="ExternalInput")
    x8d = nc.dram_tensor("x8", [3, 128, 66 * WP], dt.float8e4, kind="ExternalInput")
    w8d = nc.dram_tensor("wq8", [128, 6 * 384], dt.float8e4, kind="ExternalInput")
    outd = nc.dram_tensor("out", [3, 128, NT], dt.bfloat16, kind="ExternalOutput")
    with tile.TileContext(nc) as tc:
        _build_body(nc, tc, (xd, wqd, dtd, wpd, idd, mkd, tpd, dsd, outd,
                             x8d, w8d))
    nc.compile()
    _CACHE["nc"] = nc
    return nc


def make_in_maps(x, qkv_w, dw_w, proj_w, temperature):
    x = np.asarray(x, np.float32)
    qkv_w = np.asarray(qkv_w, np.float32)
    dw_w = np.asarray(dw_w, np.float32)
    proj_w = np.asarray(proj_w, np.float32)
    temperature = np.asarray(temperature, np.float32).reshape(-1)

    # frame layout: padded row r holds image row r-1; cols 0..127 = x, 128/129 = 0
    xpad = np.zeros((B, C, 130, 130), np.float32)
    xpad[:, :, 1:129, 0:128] = x

    wq = np.zeros((128, 9 * 384), np.float32)
    for ob in range(9):
        for ch in range(3):
            blk = qkv_w[ob * 128:(ob + 1) * 128, ch * 128:(ch + 1) * 128]
            wq[:, ob * 384 + ch * 128: ob * 384 + (ch + 1) * 128] = blk.T
    dtap = np.zeros((128, 9 * 6 * 128), np.float32)
    rng = np.arange(128)
    for ob in range(9):
        for ti, k9 in enumerate(PE_TAPS):
            col = (ob * 6 + ti) * 128
            dtap[rng, col + rng] = dw_w[ob * 128 + rng, 0, k9 // 3, k9 % 3]
    wpm = np.zeros((128, 3 * 384), np.float32)
    for ch in range(3):
        wpm[:, ch * 384:(ch + 1) * 384] = proj_w[:, ch * 128:(ch + 1) * 128].T
    ident = np.eye(128, dtype=np.float32)
    mask = np.full((128, 3 * 384), -1e30, np.float32)
    for j in range(3):
        for p in range(128):
            hgrp = (128 * j + p) // CHD
            mask[p, 384 * j + CHD * hgrp: 384 * j + CHD * (hgrp + 1)] = 0.0
    tempc = np.zeros((128, 4), np.float32)
    for j in range(3):
        for p in range(128):
            tempc[p, j] = temperature[(128 * j + p) // CHD]
    dwsc = np.zeros((128, 81), np.float32)
    for ob in range(9):
        for k9 in range(9):
            dwsc[:, ob * 9 + k9] = dw_w[ob * 128:(ob + 1) * 128, 0,
                                        k9 // 3, k9 % 3]

    F8 = ml_dtypes.float8_e4m3fn
    shared = {
        "wq": wq.astype(BF16), "dtap": dtap.astype(BF16),
        "wq8": (wq[:, :6 * 384] * 64.0).astype(F8),
        "wp": wpm.astype(BF16), "identb": ident.astype(BF16),
        "maskt": mask.astype(BF16), "tempc": tempc, "dwsc": dwsc,
    }
    in_maps = []
    for core in range(8):
        b, s = core // 2, core % 2
        xs = xpad[b, :, 64 * s: 64 * s + 66, :]          # [384, 66, 130]
        xs = np.ascontiguousarray(xs.reshape(3, 128, 66 * WP))
        m = dict(shared)
        m["x"] = xs.astype(BF16)
        m["x8"] = xs.astype(F8)
        in_maps.append(m)
    return in_maps


def assemble(results):
    full = np.zeros((B, C, 128, 128), np.float32)
    for core in range(8):
        b, s = core // 2, core % 2
        o = np.asarray(results[core]["out"], np.float32).reshape(C, 64, 128)
        full[b, :, 64 * s: 64 * s + 64, :] = o
    return full


def kernel(x, qkv_w, dw_w, proj_w, temperature):
    from concourse.bass_utils import run_bass_kernel_spmd
    nc = build_nc()
    in_maps = make_in_maps(x, qkv_w, dw_w, proj_w, temperature)
    res = run_bass_kernel_spmd(nc, in_maps, core_ids=list(range(8)))
    return assemble(res.results)
